# revision 1
# baseline (speedup 1.0000x reference)
"""Self-contained TRN2 Bass kernel for nn_Encoder_49065706389648.

Transformer encoder layer (B=8, S=2048, D=768, HID=1536), data-parallel:
one batch element per NeuronCore across 8 cores, weights replicated.
Feature-major layout (host pre-transposes), fp8 DoubleRow for the
attention block and QKV/output projections (residual-damped paths),
bf16 MLP with fp32 accumulation/residuals/LayerNorm.

kernel(**inputs) takes the FULL unsharded inputs (as from setup_inputs())
and returns the FULL [8, 2048, 768] float32 output.
"""
import sys
sys.path.insert(0, '/opt/trn_rl_repo')

# ---------------------------------------------------------------- birpatch --
# This walrus build rejects instructions carrying more than ~1-2 semaphore
# waits ("Too many sync wait commands"). Split excess waits onto injected
# wait-only EventSemaphore instructions at the BIR JSON level.
import base64
import orjson
import zstandard

MAXW = 1

_counter = [0]


def split_waits(bir: dict, maxw: int = MAXW) -> int:
    nsplit = 0
    for fn in bir.get("functions", []):
        for blk in fn.get("blocks", []):
            insts = blk.get("instructions", [])
            new_insts = []
            for ins in insts:
                si = ins.get("sync_info")
                waits = (si or {}).get("on_wait") or []
                cap = {"Drain": 0}.get(ins.get("opcode"), maxw)
                if len(waits) > cap:
                    excess, keep = (waits, []) if cap == 0 else (waits[:-cap], waits[-cap:])
                    for i in range(0, len(excess), maxw):
                        _counter[0] += 1
                        new_insts.append({
                            "engine": ins["engine"],
                            "ins": [],
                            "outs": [],
                            "name": f"wsplit-{_counter[0]}",
                            "opcode": "EventSemaphore",
                            "sync_info": {
                                "on_update": [],
                                "on_wait": excess[i:i + maxw],
                            },
                            "debug": ins.get("debug", 0),
                        })
                    si["on_wait"] = keep
                    nsplit += 1
                new_insts.append(ins)
            blk["instructions"] = new_insts
    return nsplit


def install():
    import concourse.bass2jax as bass2jax
    import concourse.bass_utils as bass_utils
    if getattr(bass2jax, "_ant_wait_split_installed", False):
        return
    import os
    if os.environ.get("ANT_LDW_OPT", "0") == "1":
        _orig_run = bass_utils.run_command
        def _run(argv, **kw):
            argv = ["--enable-ldw-opt=true" if a == "--enable-ldw-opt=false"
                    else a for a in argv]
            return _orig_run(argv, **kw)
        bass_utils.run_command = _run

    def _patched(ant_bir_value: str) -> bytes:
        raw = zstandard.ZstdDecompressor().decompress(
            base64.standard_b64decode(ant_bir_value)
        )
        bir = orjson.loads(raw)
        n = split_waits(bir)
        if n:
            print(f"[birpatch] split waits on {n} instructions")
        return orjson.dumps(bir)

    bass2jax._decompress_ant_bir = _patched
    bass2jax._ant_wait_split_installed = True


# ----------------------------------------------------------------- builder --
import numpy as np
import ml_dtypes
import concourse.bass as bass
import concourse.mybir as mybir
import concourse.tile as tile

F32 = mybir.dt.float32
BF16 = mybir.dt.bfloat16
FP8 = mybir.dt.float8e4
PM = mybir.MatmulPerfMode
AF = mybir.ActivationFunctionType
OP = mybir.AluOpType

S, D, HID = 2048, 768, 1536
KD = D // 128      # 6   d-tiles
KH = HID // 128    # 12  hid-tiles
NK = S // 128      # 16  seq-tiles
CW = 512           # q-chunk width processed per pipeline pass
NQ = S // CW       # 4   chunks
EPS = 1e-12
ISCALE = float(1.0 / np.sqrt(D))
INV_D = float(1.0 / D)
INV_SQD = float(1.0 / np.sqrt(D))


class Ctx:
    pass


def build(nc: bass.Bass, reps: int = 1, loop_n: int = 0):
    c = Ctx()
    c.nc = nc
    # ---- DRAM I/O ----
    c.xt_d = nc.dram_tensor("xt", [KD // 2, 128, 2, S], FP8,
                            kind="ExternalInput")
    c.xres_d = nc.dram_tensor("xres", [KD, 128, S], F32, kind="ExternalInput")
    c.wq_d = nc.dram_tensor("wq", [KD // 2, 128, 2, D], FP8,
                            kind="ExternalInput")
    c.wk_d = nc.dram_tensor("wk", [KD // 2, 128, 2, D], FP8,
                            kind="ExternalInput")
    c.wv_d = nc.dram_tensor("wv", [KD // 2, 128, 2, D], FP8,
                            kind="ExternalInput")
    c.wp_d = nc.dram_tensor("wp", [KD // 2, 128, 2, D], FP8,
                            kind="ExternalInput")
    c.w1_d = nc.dram_tensor("w1", [KD, 128, HID], BF16, kind="ExternalInput")
    c.w2_d = nc.dram_tensor("w2", [KH, 128, D], BF16, kind="ExternalInput")
    c.bq_d = nc.dram_tensor("bq", [128, KD], F32, kind="ExternalInput")
    c.bk_d = nc.dram_tensor("bk", [128, KD], F32, kind="ExternalInput")
    c.bvr_d = nc.dram_tensor("bvr", [128, D], BF16, kind="ExternalInput")
    c.b1_d = nc.dram_tensor("b1", [128, KH], F32, kind="ExternalInput")
    c.b2_d = nc.dram_tensor("b2", [128, KD], F32, kind="ExternalInput")
    c.yt_d = nc.dram_tensor("yt", [KD, 128, S], F32, kind="ExternalOutput")

    with tile.TileContext(nc) as tc:
        with tc.tile_pool(name="sb", bufs=1) as sb, \
             tc.tile_pool(name="ps", bufs=4, space=bass.MemorySpace.PSUM) as ps:
            c.tc, c.sb, c.ps = tc, sb, ps

            c.ones = sb.tile([128, 128], BF16, tag="ones")
            nc.vector.memset(c.ones[:], 1.0)
            c.ones8 = sb.tile([128, 2, 128], FP8, tag="ones8")
            nc.vector.memset(c.ones8[:], 1.0)
            c.eps_t = sb.tile([128, 1], F32, tag="eps")
            nc.vector.memset(c.eps_t[:], EPS)

            c.bq_t = sb.tile([128, KD], F32, tag="bias", bufs=4)
            c.bk_t = sb.tile([128, KD], F32, tag="bias", bufs=4)
            c.b1_t = sb.tile([128, KH], F32, tag="bias", bufs=4)
            c.b2_t = sb.tile([128, KD], F32, tag="bias", bufs=4)
            c.bvr_t = sb.tile([128, D], BF16, tag="bvr")
            nc.sync.dma_start(c.bq_t[:], c.bq_d[:])
            nc.sync.dma_start(c.bk_t[:], c.bk_d[:])
            nc.sync.dma_start(c.b1_t[:], c.b1_d[:])
            nc.sync.dma_start(c.b2_t[:], c.b2_d[:])
            nc.sync.dma_start(c.bvr_t[:], c.bvr_d[:])

            if loop_n:
                with tc.For_i(0, loop_n, 1) as _i:
                    _pipeline(c)
            else:
                for _ in range(reps):
                    _pipeline(c)
    return nc


def _pipeline(c):
    _phase_a(c)
    st = [Ctx() for _ in range(NQ)]   # per-chunk state
    _scores(c, st[0], 0)
    _denom(c, st[0], 0)
    _attnv(c, st[0], 0)
    _proj(c, st[0], 0)
    _ln1_sums(c, st[0], 0)
    for q in range(NQ):
        if q + 1 < NQ:
            _scores(c, st[q + 1], q + 1, mids={
                2: (lambda qq=q: _ln1_stats(c, st[qq], qq)),
                6: (lambda qq=q: _ln1_apply(c, st[qq], qq)),
            })
            _denom(c, st[q + 1], q + 1)
        else:
            _ln1_stats(c, st[q], q)
            _ln1_apply(c, st[q], q)
        _mlp1(c, st[q], q)
        _mlp2(c, st[q], q)
        _ln2_sums(c, st[q], q)
        if q + 1 < NQ:
            _attnv(c, st[q + 1], q + 1, mids={
                2: (lambda qq=q: _ln2_stats(c, st[qq], qq)),
            })
            _proj(c, st[q + 1], q + 1)
            _ln2_apply(c, st[q], q)
            _ln1_sums(c, st[q + 1], q + 1)
        else:
            _ln2_stats(c, st[q], q)
            _ln2_apply(c, st[q], q)


def _phase_a(c):
    nc, sb, ps = c.nc, c.sb, c.ps
    c.xt = [sb.tile([128, 2, S], FP8, tag="xt", bufs=KD // 2, name=f"xt{i}")
            for i in range(KD // 2)]
    for i in range(KD // 2):
        nc.sync.dma_start(c.xt[i][:, :, 0:S // 2], c.xt_d[i][:, :, 0:S // 2])
        nc.sync.dma_start(c.xt[i][:, :, S // 2:S], c.xt_d[i][:, :, S // 2:S])

    wq = [sb.tile([128, 2, D], FP8, tag="w8", bufs=9, name=f"wq{i}")
          for i in range(KD // 2)]
    wk = [sb.tile([128, 2, D], FP8, tag="w8", bufs=9, name=f"wk{i}")
          for i in range(KD // 2)]
    for i in range(KD // 2):
        nc.sync.dma_start(wq[i][:], c.wq_d[i])
        nc.sync.dma_start(wk[i][:], c.wk_d[i])

    c.QT = [sb.tile([128, 2, S], FP8, tag="qkt", bufs=6, name=f"QT{i}")
            for i in range(KD // 2)]
    c.KT = [sb.tile([128, 2, S], FP8, tag="qkt", bufs=6, name=f"KT{i}")
            for i in range(KD // 2)]
    c.V = [sb.tile([128, 2, D], FP8, tag="v768", bufs=NK // 2, name=f"V{i}")
           for i in range(NK // 2)]

    with nc.named_scope("qk_proj"):
        for (W, BIAS, OUT) in ((wq, c.bq_t, c.QT), (wk, c.bk_t, c.KT)):
            for e in range(KD):
                for qh in range(2):
                    pq = ps.tile([128, 2 * CW], F32, tag="mm",
                                 name=f"pq{e}_{qh}")
                    for k in range(KD // 2):
                        for qc in range(2):
                            nc.tensor.matmul(
                                pq[:, qc * CW:(qc + 1) * CW],
                                W[k][:, :, e * 128:(e + 1) * 128],
                                c.xt[k][:, :, (2 * qh + qc) * CW:
                                        (2 * qh + qc + 1) * CW],
                                start=(k == 0), stop=(k == KD // 2 - 1),
                                perf_mode=PM.DoubleRow)
                    nc.scalar.activation(
                        OUT[e // 2][:, e % 2, qh * 2 * CW:(qh + 1) * 2 * CW],
                        pq[:], AF.Identity, bias=BIAS[:, e:e + 1])

    wv = [sb.tile([128, 2, D], FP8, tag="w8", bufs=9, name=f"wv{i}")
          for i in range(KD // 2)]
    for i in range(KD // 2):
        nc.sync.dma_start(wv[i][:], c.wv_d[i])

    with nc.named_scope("v_proj"):
        for s in range(NK):
            pv = ps.tile([128, 2 * CW], F32, tag="mm", name=f"pv{s}")
            for k in range(KD // 2):
                nc.tensor.matmul(pv[:, 0:512],
                                 c.xt[k][:, :, s * 128:(s + 1) * 128],
                                 wv[k][:, :, 0:512],
                                 start=(k == 0), stop=(k == KD // 2 - 1),
                                 perf_mode=PM.DoubleRow)
                nc.tensor.matmul(pv[:, 512:768],
                                 c.xt[k][:, :, s * 128:(s + 1) * 128],
                                 wv[k][:, :, 512:768],
                                 start=(k == 0), stop=(k == KD // 2 - 1),
                                 perf_mode=PM.DoubleRow)
            nc.vector.tensor_tensor(c.V[s // 2][:, s % 2, :], pv[:, 0:D],
                                    c.bvr_t[:], op=OP.add)

    c.wp = [sb.tile([128, 2, D], FP8, tag="wp", bufs=KD // 2, name=f"wp{i}")
            for i in range(KD // 2)]
    c.w1 = [sb.tile([128, HID], BF16, tag="w1536", bufs=KD, name=f"w1{i}")
            for i in range(KD)]
    c.w2 = [sb.tile([128, D], BF16, tag="w768", bufs=12, name=f"w2{i}")
            for i in range(KH)]
    for i in range(KD // 2):
        nc.sync.dma_start(c.wp[i][:], c.wp_d[i])
    for i in range(KD):
        nc.sync.dma_start(c.w1[i][:], c.w1_d[i])
    for i in range(KH):
        nc.sync.dma_start(c.w2[i][:], c.w2_d[i])


def _scores(c, s, q, mids=None):
    nc, sb, ps = c.nc, c.sb, c.ps
    cs = slice(q * CW, (q + 1) * CW)
    s.PT = [sb.tile([128, 2, CW], FP8, tag="pt", bufs=10,
                    name=f"PT{q}_{k}") for k in range(NK // 2)]
    with nc.named_scope(f"scores{q}"):
        for k in range(NK):
            if mids and k in mids:
                mids[k]()
            pss = ps.tile([128, 2 * CW], F32, tag="mm", name=f"pss{q}_{k}")
            for i in range(KD // 2):
                nc.tensor.matmul(pss[:, 0:CW],
                                 c.KT[i][:, :, k * 128:(k + 1) * 128],
                                 c.QT[i][:, :, cs],
                                 start=(i == 0), stop=(i == KD // 2 - 1),
                                 perf_mode=PM.DoubleRow)
            pt_half = s.PT[k // 2][:, k % 2, :]
            nc.scalar.activation(pt_half, pss[:, 0:CW], AF.Exp, scale=ISCALE)


def _denom(c, s, q):
    nc, sb, ps = c.nc, c.sb, c.ps
    s.rden = sb.tile([128, CW], F32, tag="f32c", bufs=22, name=f"rden{q}")
    with nc.named_scope(f"denom{q}"):
        psd = ps.tile([128, 2 * CW], F32, tag="mm", name=f"psd{q}")
        for j in range(NK // 2):
            nc.tensor.matmul(psd[:, 0:CW], c.ones8[:], s.PT[j][:, :, :],
                             start=(j == 0), stop=(j == NK // 2 - 1),
                             perf_mode=PM.DoubleRow)
        nc.vector.reciprocal(s.rden[:], psd[:, 0:CW])


def _attnv(c, s, q, mids=None):
    nc, sb, ps = c.nc, c.sb, c.ps
    s.attnT = [sb.tile([128, 2, CW], FP8, tag="attc", bufs=12,
                       name=f"at{q}_{d}") for d in range(KD // 2)]
    with nc.named_scope(f"attnv{q}"):
        for d in range(KD):
            if mids and d in mids:
                mids[d]()
            pa = ps.tile([128, 2 * CW], F32, tag="mm", name=f"pa{q}_{d}")
            for j in range(NK // 2):
                nc.tensor.matmul(pa[:, 0:CW],
                                 c.V[j][:, :, d * 128:(d + 1) * 128],
                                 s.PT[j][:, :, :],
                                 start=(j == 0), stop=(j == NK // 2 - 1),
                                 perf_mode=PM.DoubleRow)
            nc.vector.tensor_tensor(s.attnT[d // 2][:, d % 2, :],
                                    pa[:, 0:CW], s.rden[:], op=OP.mult)


def _proj(c, s, q):
    nc, sb, ps = c.nc, c.sb, c.ps
    cs = slice(q * CW, (q + 1) * CW)
    xres = [sb.tile([128, CW], F32, tag="f32c", bufs=22, name=f"xr{q}_{e}")
            for e in range(KD)]
    for e in range(KD):
        nc.sync.dma_start(xres[e][:], c.xres_d[e][:, cs])
    s.z = [sb.tile([128, CW], F32, tag="f32c", bufs=22, name=f"z{q}_{e}")
           for e in range(KD)]
    with nc.named_scope(f"proj{q}"):
        for e in range(KD):
            pp = ps.tile([128, 2 * CW], F32, tag="mm", name=f"pp{q}_{e}")
            for d in range(KD // 2):
                nc.tensor.matmul(pp[:, 0:CW],
                                 c.wp[d][:, :, e * 128:(e + 1) * 128],
                                 s.attnT[d][:, :, :],
                                 start=(d == 0), stop=(d == KD // 2 - 1),
                                 perf_mode=PM.DoubleRow)
            nc.vector.tensor_tensor(s.z[e][:], pp[:, 0:CW], xres[e][:],
                                    op=OP.add)


def _ln_sums(c, s, q, which, z):
    """sums: psum[:, :CW] = mean(z), psum[:, CW:2CW] = mean(z^2), both
    partition-replicated. Tile sums run on DVE (f32), partition reduction is
    one ones-matmul each."""
    nc, sb, ps = c.nc, c.sb, c.ps
    with nc.named_scope(f"ln{which}s_{q}"):
        zacc = sb.tile([128, CW], F32, tag="acc2", bufs=2,
                       name=f"zacc{q}_{which}")
        sacc = sb.tile([128, CW], F32, tag="acc2", bufs=2,
                       name=f"sacc{q}_{which}")
        sq = [sb.tile([128, CW], BF16, tag="sq", bufs=3,
                      name=f"sq{q}_{which}_{e}") for e in range(KD)]
        for e in range(KD):
            nc.scalar.activation(sq[e][:], z[e][:], AF.Square, scale=INV_SQD)
            if e == 1:
                nc.gpsimd.tensor_tensor(zacc[:], z[0][:], z[1][:], op=OP.add)
                nc.vector.tensor_tensor(sacc[:], sq[0][:], sq[1][:], op=OP.add)
            elif e > 1:
                nc.gpsimd.tensor_tensor(zacc[:], zacc[:], z[e][:], op=OP.add)
                nc.vector.tensor_tensor(sacc[:], sacc[:], sq[e][:], op=OP.add)
        zb = sb.tile([128, CW], BF16, tag="zb", bufs=2,
                     name=f"zb{q}_{which}")
        sb2 = sb.tile([128, CW], BF16, tag="sq", bufs=3,
                      name=f"sb{q}_{which}")
        nc.vector.tensor_scalar(zb[:], zacc[:], INV_D, None, op0=OP.mult)
        nc.vector.tensor_copy(sb2[:], sacc[:])
    s.__setattr__(f"lnzb{which}", (zb, sb2))
    return None


def _ln_sums_mm(c, s, q, which, _unused):
    nc, ps = c.nc, c.ps
    zb, sb2 = getattr(s, f"lnzb{which}")
    pst = ps.tile([128, 2 * CW], F32, tag="mm", name=f"pst{q}_{which}")
    with nc.named_scope(f"ln{which}m_{q}"):
        nc.tensor.matmul(pst[:, 0:CW], c.ones[:], zb[:], start=True, stop=True)
        nc.tensor.matmul(pst[:, CW:2 * CW], c.ones[:], sb2[:],
                         start=True, stop=True)
    if which == 1:
        s.pst1 = pst
    else:
        s.pst2 = pst


def _ln_stats(c, s, q, which, pst):
    """From sums psum -> (M=mean, Sv=mean*rstd, t0=rstd), all replicated."""
    nc, sb = c.nc, c.sb
    M = sb.tile([128, CW], F32, tag="f32c", bufs=22, name=f"M{q}_{which}")
    Sv = sb.tile([128, CW], F32, tag="f32c", bufs=22, name=f"S{q}_{which}")
    t0 = sb.tile([128, CW], F32, tag="f32c", bufs=22, name=f"t0{q}_{which}")
    with nc.named_scope(f"ln{which}st_{q}"):
        nc.scalar.activation(t0[:], pst[:, 0:CW], AF.Square)      # mu^2
        nc.vector.tensor_copy(M[:], pst[:, 0:CW])                 # mu
        nc.vector.tensor_tensor(Sv[:], pst[:, CW:2 * CW], t0[:],
                                op=OP.subtract)                   # var
        nc.scalar.activation(Sv[:], Sv[:], AF.Sqrt, bias=c.eps_t[:])
        nc.vector.reciprocal(t0[:], Sv[:])                        # rstd
        nc.vector.tensor_tensor(Sv[:], M[:], t0[:], op=OP.mult)   # mu*rstd
    return M, Sv, t0


def _ln1_sums(c, s, q):
    s.pst1 = _ln_sums(c, s, q, 1, s.z)


def _ln1_stats(c, s, q):
    _ln_sums_mm(c, s, q, 1, None)
    s.st1 = _ln_stats(c, s, q, 1, s.pst1)


def _ln1_apply(c, s, q):
    nc, sb = c.nc, c.sb
    M, Sv, t0 = s.st1
    s.hb = [sb.tile([128, CW], BF16, tag="attc", bufs=12, name=f"hb{q}_{e}")
            for e in range(KD)]
    with nc.named_scope(f"ln1a_{q}"):
        for e in range(KD):
            nc.vector.tensor_tensor(s.z[e][:], s.z[e][:], t0[:], op=OP.mult)
            nc.vector.tensor_tensor(s.hb[e][:], s.z[e][:], Sv[:],
                                    op=OP.subtract)
        # f32 residual carrier: hres = z*rstd - mu*rstd + b2 (in place, GP)
        for e in range(KD):
            nc.gpsimd.tensor_tensor(s.z[e][:], s.z[e][:], Sv[:],
                                    op=OP.subtract)
            nc.gpsimd.tensor_scalar(s.z[e][:], s.z[e][:],
                                    c.b2_t[:, e:e + 1], None, op0=OP.add)
    s.hres = s.z


def _mlp1(c, s, q):
    nc, sb, ps = c.nc, c.sb, c.ps
    s.mlpb = [sb.tile([128, CW], BF16, tag="mlpb", bufs=KH, name=f"mb{q}_{h}")
              for h in range(KH)]
    with nc.named_scope(f"mlp1_{q}"):
        for h in range(KH):
            pm = ps.tile([128, 2 * CW], F32, tag="mm", name=f"pm{q}_{h}")
            for k in range(KD):
                nc.tensor.matmul(pm[:, 0:CW],
                                 c.w1[k][:, h * 128:(h + 1) * 128],
                                 s.hb[k][:],
                                 start=(k == 0), stop=(k == KD - 1))
            nc.scalar.activation(s.mlpb[h][:], pm[:, 0:CW], AF.Gelu,
                                 bias=c.b1_t[:, h:h + 1])


def _mlp2(c, s, q):
    nc, sb, ps = c.nc, c.sb, c.ps
    with nc.named_scope(f"mlp2_{q}"):
        for e in range(KD):
            p2 = ps.tile([128, 2 * CW], F32, tag="mm", name=f"p2{q}_{e}")
            for k in range(KH):
                nc.tensor.matmul(p2[:, 0:CW],
                                 c.w2[k][:, e * 128:(e + 1) * 128],
                                 s.mlpb[k][:],
                                 start=(k == 0), stop=(k == KH - 1))
            nc.vector.tensor_tensor(s.hres[e][:], p2[:, 0:CW], s.hres[e][:],
                                    op=OP.add)
    s.z2 = s.hres


def _ln2_sums(c, s, q):
    s.pst2 = _ln_sums(c, s, q, 2, s.z2)


def _ln2_stats(c, s, q):
    _ln_sums_mm(c, s, q, 2, None)
    s.st2 = _ln_stats(c, s, q, 2, s.pst2)


def _ln2_apply(c, s, q):
    nc = c.nc
    cs = slice(q * CW, (q + 1) * CW)
    M2, Sv2, t02 = s.st2
    with nc.named_scope(f"ln2a_{q}"):
        for e in range(KD):
            eng = nc.vector if e < 3 else nc.gpsimd
            eng.tensor_tensor(s.z2[e][:], s.z2[e][:], t02[:], op=OP.mult)
            eng.tensor_tensor(s.z2[e][:], s.z2[e][:], Sv2[:],
                              op=OP.subtract)
            nc.sync.dma_start(c.yt_d[e][:, cs], s.z2[e][:])


# ---------------- host side ----------------

def host_prep(inputs):
    """Returns per-core input maps (weights shared)."""
    bf = ml_dtypes.bfloat16
    x = np.asarray(inputs["x"], np.float32)
    B = x.shape[0]

    f8 = ml_dtypes.float8_e4m3

    def wtile(w, kt):  # [out,in] -> transposed, tiled on contraction dim
        wt = np.ascontiguousarray(np.asarray(w, np.float32).T)  # [in, out]
        return wt.reshape(kt, 128, wt.shape[1]).astype(bf)

    def wtile8(w, kt):  # fp8 DoubleRow pairs: [kt//2, 128, 2, out]
        wt = np.ascontiguousarray(np.asarray(w, np.float32).T)
        t = wt.reshape(kt // 2, 2, 128, wt.shape[1]).transpose(0, 2, 1, 3)
        return np.ascontiguousarray(t).astype(f8)

    shared = {
        "wq": wtile8(inputs["Wq"], KD), "wk": wtile8(inputs["Wk"], KD),
        "wv": wtile8(inputs["Wv"], KD), "wp": wtile8(inputs["Wp"], KD),
        "w1": wtile(inputs["W1"], KD), "w2": wtile(inputs["W2"], KH),
        "bq": np.ascontiguousarray(
            np.asarray(inputs["bq"], np.float32).reshape(KD, 128).T),
        "bk": np.ascontiguousarray(
            np.asarray(inputs["bk"], np.float32).reshape(KD, 128).T),
        "b1": np.ascontiguousarray(
            np.asarray(inputs["b1"], np.float32).reshape(KH, 128).T),
        "b2": np.ascontiguousarray(
            np.asarray(inputs["b2"], np.float32).reshape(KD, 128).T),
        "bvr": np.ascontiguousarray(
            np.broadcast_to(np.asarray(inputs["bv"], np.float32).astype(bf),
                            (128, D))),
    }
    bp = np.asarray(inputs["bp"], np.float32)
    per_core = []
    for b in range(B):
        xb_t = np.ascontiguousarray(x[b].T)          # [D, S]
        m = dict(shared)
        m["xt"] = np.ascontiguousarray(
            xb_t.reshape(KD // 2, 2, 128, S).transpose(0, 2, 1, 3)).astype(f8)
        m["xres"] = (xb_t + bp[:, None]).reshape(KD, 128, S).astype(np.float32)
        per_core.append(m)
    return per_core


def assemble_output(results):
    """results: list of per-core dicts with 'yt' [KD,128,S] -> [B,S,D] f32."""
    B = len(results)
    out = np.empty((B, S, D), np.float32)
    for b in range(B):
        out[b] = results[b]["yt"].reshape(D, S).T
    return out


# ------------------------------------------------------------------ kernel --
_CACHE = {}


def kernel(**inputs):
    install()  # birpatch
    from concourse.bass_utils import run_bass_kernel_spmd

    per_core = host_prep(inputs)
    n = len(per_core)
    key = "nc%d" % n
    if key not in _CACHE:
        _nc = bass.Bass("TRN2", target_bir_lowering=False, debug=False,
                        num_devices=n)
        build(_nc, reps=1)
        _CACHE[key] = _nc
    _nc = _CACHE[key]
    res = run_bass_kernel_spmd(_nc, per_core, list(range(n)), trace=False)
    return assemble_output(res.results)



# revision 5
# speedup vs baseline: 1.2928x; 1.2928x over previous
"""Self-contained TRN2 Bass kernel for nn_Encoder_49065706389648.

Transformer encoder layer (B=8, S=2048, D=768, HID=1536), data-parallel:
one batch element per NeuronCore across 8 cores, weights replicated.
Feature-major layout (host pre-transposes), fp8 DoubleRow for ALL matmuls
(QKV/attention/proj and the MLP; W1/W2 are x32-scaled into fp8 normal
range, compensated exactly via the Gelu activation scale and a x32-scaled
residual carrier — LayerNorm is scale-invariant), fp32 accumulation,
residuals and LayerNorm stats.

kernel(**inputs) takes the FULL unsharded inputs (as from setup_inputs())
and returns the FULL [8, 2048, 768] float32 output.
"""
import sys
sys.path.insert(0, '/opt/trn_rl_repo')

# ---------------------------------------------------------------- birpatch --
# This walrus build rejects instructions carrying more than ~1-2 semaphore
# waits ("Too many sync wait commands"). Split excess waits onto injected
# wait-only EventSemaphore instructions at the BIR JSON level.
import base64
import orjson
import zstandard

MAXW = 1

_counter = [0]


def split_waits(bir: dict, maxw: int = MAXW) -> int:
    nsplit = 0
    for fn in bir.get("functions", []):
        for blk in fn.get("blocks", []):
            insts = blk.get("instructions", [])
            new_insts = []
            for ins in insts:
                si = ins.get("sync_info")
                waits = (si or {}).get("on_wait") or []
                cap = {"Drain": 0}.get(ins.get("opcode"), maxw)
                if len(waits) > cap:
                    excess, keep = (waits, []) if cap == 0 else (waits[:-cap], waits[-cap:])
                    for i in range(0, len(excess), maxw):
                        _counter[0] += 1
                        new_insts.append({
                            "engine": ins["engine"],
                            "ins": [],
                            "outs": [],
                            "name": f"wsplit-{_counter[0]}",
                            "opcode": "EventSemaphore",
                            "sync_info": {
                                "on_update": [],
                                "on_wait": excess[i:i + maxw],
                            },
                            "debug": ins.get("debug", 0),
                        })
                    si["on_wait"] = keep
                    nsplit += 1
                new_insts.append(ins)
            blk["instructions"] = new_insts
    return nsplit


def install():
    import concourse.bass2jax as bass2jax
    import concourse.bass_utils as bass_utils
    if getattr(bass2jax, "_ant_wait_split_installed", False):
        return
    import os
    if os.environ.get("ANT_LDW_OPT", "0") == "1":
        _orig_run = bass_utils.run_command
        def _run(argv, **kw):
            argv = ["--enable-ldw-opt=true" if a == "--enable-ldw-opt=false"
                    else a for a in argv]
            return _orig_run(argv, **kw)
        bass_utils.run_command = _run

    def _patched(ant_bir_value: str) -> bytes:
        raw = zstandard.ZstdDecompressor().decompress(
            base64.standard_b64decode(ant_bir_value)
        )
        bir = orjson.loads(raw)
        n = split_waits(bir)
        if n:
            print(f"[birpatch] split waits on {n} instructions")
        return orjson.dumps(bir)

    bass2jax._decompress_ant_bir = _patched
    bass2jax._ant_wait_split_installed = True


# ----------------------------------------------------------------- builder --
import numpy as np
import ml_dtypes
import concourse.bass as bass
import concourse.mybir as mybir
import concourse.tile as tile

F32 = mybir.dt.float32
BF16 = mybir.dt.bfloat16
FP8 = mybir.dt.float8e4
PM = mybir.MatmulPerfMode
AF = mybir.ActivationFunctionType
OP = mybir.AluOpType

S, D, HID = 2048, 768, 1536
KD = D // 128      # 6   d-tiles
KH = HID // 128    # 12  hid-tiles
NK = S // 128      # 16  seq-tiles
CW = 512           # q-chunk width processed per pipeline pass
NQ = S // CW       # 4   chunks
EPS = 1e-12
ISCALE = float(1.0 / np.sqrt(D))
INV_D = float(1.0 / D)
INV_SQD = float(1.0 / np.sqrt(D))
WSC = 32.0         # fp8 scale for W1/W2 (and the LN1 residual carrier)


def bc2(pair_ap, small_ap):
    """Broadcast a [128, CW] AP across the middle dim of a [128, 2, CW] AP."""
    _, b = bass.broadcast_tensor_aps(pair_ap, small_ap)
    return b


class Ctx:
    pass


def build(nc: bass.Bass, reps: int = 1, loop_n: int = 0):
    c = Ctx()
    c.nc = nc
    # ---- DRAM I/O ----
    c.xt_d = nc.dram_tensor("xt", [KD // 2, 128, 2, S], FP8,
                            kind="ExternalInput")
    c.xres_d = nc.dram_tensor("xres", [KD // 2, 128, 2, S], F32,
                              kind="ExternalInput")
    c.wq_d = nc.dram_tensor("wq", [KD // 2, 128, 2, D], FP8,
                            kind="ExternalInput")
    c.wk_d = nc.dram_tensor("wk", [KD // 2, 128, 2, D], FP8,
                            kind="ExternalInput")
    c.wv_d = nc.dram_tensor("wv", [KD // 2, 128, 2, D], FP8,
                            kind="ExternalInput")
    c.wp_d = nc.dram_tensor("wp", [KD // 2, 128, 2, D], FP8,
                            kind="ExternalInput")
    c.w1_d = nc.dram_tensor("w1", [KD // 2, 128, 2, HID], FP8,
                            kind="ExternalInput")
    c.w2_d = nc.dram_tensor("w2", [KH // 2, 128, 2, D], FP8,
                            kind="ExternalInput")
    c.bq_d = nc.dram_tensor("bq", [128, KD], F32, kind="ExternalInput")
    c.bk_d = nc.dram_tensor("bk", [128, KD], F32, kind="ExternalInput")
    c.b1_d = nc.dram_tensor("b1", [128, KH], F32, kind="ExternalInput")
    c.b2_d = nc.dram_tensor("b2", [128, KD], F32, kind="ExternalInput")
    c.yt_d = nc.dram_tensor("yt", [KD // 2, 128, 2, S], F32,
                            kind="ExternalOutput")

    with tile.TileContext(nc) as tc:
        with tc.tile_pool(name="sb", bufs=1) as sb, \
             tc.tile_pool(name="ps", bufs=4, space=bass.MemorySpace.PSUM) as ps:
            c.tc, c.sb, c.ps = tc, sb, ps

            c.ones = sb.tile([128, 128], BF16, tag="ones")
            nc.vector.memset(c.ones[:], 1.0)
            c.ones8 = sb.tile([128, 2, 128], FP8, tag="ones8")
            nc.vector.memset(c.ones8[:], 1.0)
            c.eps_t = sb.tile([128, 1], F32, tag="eps")
            nc.vector.memset(c.eps_t[:], EPS)

            c.bq_t = sb.tile([128, KD], F32, tag="bias", bufs=4)
            c.bk_t = sb.tile([128, KD], F32, tag="bias", bufs=4)
            c.b1_t = sb.tile([128, KH], F32, tag="bias", bufs=4)
            c.b2_t = sb.tile([128, KD], F32, tag="bias", bufs=4)
            nc.sync.dma_start(c.bq_t[:], c.bq_d[:])
            nc.sync.dma_start(c.bk_t[:], c.bk_d[:])
            nc.sync.dma_start(c.b1_t[:], c.b1_d[:])
            nc.sync.dma_start(c.b2_t[:], c.b2_d[:])

            if loop_n:
                with tc.For_i(0, loop_n, 1) as _i:
                    _pipeline(c)
            else:
                for _ in range(reps):
                    _pipeline(c)
    return nc


def _pipeline(c):
    _phase_a(c)
    st = [Ctx() for _ in range(NQ)]   # per-chunk state
    _scores(c, st[0], 0)
    _denom(c, st[0], 0)
    _attnv(c, st[0], 0)
    _proj(c, st[0], 0)
    _ln1_sums(c, st[0], 0)
    for q in range(NQ):
        if q + 1 < NQ:
            _scores(c, st[q + 1], q + 1, mids={
                2: (lambda qq=q: _ln1_stats(c, st[qq], qq)),
                6: (lambda qq=q: _ln1_apply(c, st[qq], qq)),
            })
            _denom(c, st[q + 1], q + 1)
        else:
            _ln1_stats(c, st[q], q)
            _ln1_apply(c, st[q], q)
        _mlp1(c, st[q], q)
        _mlp2(c, st[q], q)
        _ln2_sums(c, st[q], q)
        if q + 1 < NQ:
            _attnv(c, st[q + 1], q + 1, mids={
                2: (lambda qq=q: _ln2_stats(c, st[qq], qq)),
            })
            _proj(c, st[q + 1], q + 1)
            _ln2_apply(c, st[q], q)
            _ln1_sums(c, st[q + 1], q + 1)
        else:
            _ln2_stats(c, st[q], q)
            _ln2_apply(c, st[q], q)


def _phase_a(c):
    nc, sb, ps = c.nc, c.sb, c.ps
    c.xt = [sb.tile([128, 2, S], FP8, tag="xt", bufs=KD // 2, name=f"xt{i}")
            for i in range(KD // 2)]
    for i in range(KD // 2):
        nc.sync.dma_start(c.xt[i][:, :, 0:S // 2], c.xt_d[i][:, :, 0:S // 2])
        nc.sync.dma_start(c.xt[i][:, :, S // 2:S], c.xt_d[i][:, :, S // 2:S])

    wq = [sb.tile([128, 2, D], FP8, tag="w8", bufs=9, name=f"wq{i}")
          for i in range(KD // 2)]
    wk = [sb.tile([128, 2, D], FP8, tag="w8", bufs=9, name=f"wk{i}")
          for i in range(KD // 2)]
    for i in range(KD // 2):
        nc.sync.dma_start(wq[i][:], c.wq_d[i])
        nc.sync.dma_start(wk[i][:], c.wk_d[i])

    c.QT = [sb.tile([128, 2, S], FP8, tag="qkt", bufs=6, name=f"QT{i}")
            for i in range(KD // 2)]
    c.KT = [sb.tile([128, 2, S], FP8, tag="qkt", bufs=6, name=f"KT{i}")
            for i in range(KD // 2)]
    c.V = [sb.tile([128, 2, D], FP8, tag="v768", bufs=NK // 2, name=f"V{i}")
           for i in range(NK // 2)]

    with nc.named_scope("qk_proj"):
        for wi, (W, BIAS, OUT) in enumerate(
                ((wq, c.bq_t, c.QT), (wk, c.bk_t, c.KT))):
            for e in range(KD):
                for qh in range(2):
                    pq = ps.tile([128, 2 * CW], F32, tag="mm",
                                 name=f"pq{e}_{qh}")
                    for k in range(KD // 2):
                        for qc in range(2):
                            nc.tensor.matmul(
                                pq[:, qc * CW:(qc + 1) * CW],
                                W[k][:, :, e * 128:(e + 1) * 128],
                                c.xt[k][:, :, (2 * qh + qc) * CW:
                                        (2 * qh + qc + 1) * CW],
                                start=(k == 0), stop=(k == KD // 2 - 1),
                                perf_mode=PM.DoubleRow)
                    dst = OUT[e // 2][:, e % 2, qh * 2 * CW:(qh + 1) * 2 * CW]
                    if (e + qh + wi) % 2 == 0:
                        nc.scalar.activation(dst, pq[:], AF.Identity,
                                             bias=BIAS[:, e:e + 1])
                    else:
                        nc.vector.tensor_scalar(dst, pq[:], BIAS[:, e:e + 1],
                                                None, op0=OP.add)

    wv = [sb.tile([128, 2, D], FP8, tag="w8", bufs=9, name=f"wv{i}")
          for i in range(KD // 2)]
    for i in range(KD // 2):
        nc.sync.dma_start(wv[i][:], c.wv_d[i])

    with nc.named_scope("v_proj"):
        for s in range(NK):
            pv = ps.tile([128, 2 * CW], F32, tag="mm", name=f"pv{s}")
            for k in range(KD // 2):
                nc.tensor.matmul(pv[:, 0:512],
                                 c.xt[k][:, :, s * 128:(s + 1) * 128],
                                 wv[k][:, :, 0:512],
                                 start=(k == 0), stop=(k == KD // 2 - 1),
                                 perf_mode=PM.DoubleRow)
                nc.tensor.matmul(pv[:, 512:768],
                                 c.xt[k][:, :, s * 128:(s + 1) * 128],
                                 wv[k][:, :, 512:768],
                                 start=(k == 0), stop=(k == KD // 2 - 1),
                                 perf_mode=PM.DoubleRow)
            # bv is folded into xres on the host (exact: softmax rows sum
            # to 1 so attn(V+bv) = attn(V) + bv, and Wp@bv joins bp).
            if s % 2 == 0:
                nc.scalar.activation(c.V[s // 2][:, s % 2, :], pv[:, 0:D],
                                     AF.Identity)
            else:
                nc.vector.tensor_copy(c.V[s // 2][:, s % 2, :], pv[:, 0:D])

    c.wp = [sb.tile([128, 2, D], FP8, tag="wp", bufs=KD // 2, name=f"wp{i}")
            for i in range(KD // 2)]
    c.w1 = [sb.tile([128, 2, HID], FP8, tag="w1536", bufs=KD // 2,
                    name=f"w1{i}") for i in range(KD // 2)]
    c.w2 = [sb.tile([128, 2, D], FP8, tag="w768", bufs=KH // 2,
                    name=f"w2{i}") for i in range(KH // 2)]
    for i in range(KD // 2):
        nc.sync.dma_start(c.wp[i][:], c.wp_d[i])
    for i in range(KD // 2):
        nc.sync.dma_start(c.w1[i][:], c.w1_d[i])
    for i in range(KH // 2):
        nc.sync.dma_start(c.w2[i][:], c.w2_d[i])


def _scores(c, s, q, mids=None):
    nc, sb, ps = c.nc, c.sb, c.ps
    cs = slice(q * CW, (q + 1) * CW)
    s.PT = [sb.tile([128, 2, CW], FP8, tag="pt", bufs=10,
                    name=f"PT{q}_{k}") for k in range(NK // 2)]
    with nc.named_scope(f"scores{q}"):
        for kp in range(NK // 2):
            pss = ps.tile([128, 2, CW], F32, tag="mm", name=f"pss{q}_{kp}")
            for half in range(2):
                k = 2 * kp + half
                if mids and k in mids:
                    mids[k]()
                for i in range(KD // 2):
                    nc.tensor.matmul(pss[:, half, :],
                                     c.KT[i][:, :, k * 128:(k + 1) * 128],
                                     c.QT[i][:, :, cs],
                                     start=(i == 0), stop=(i == KD // 2 - 1),
                                     perf_mode=PM.DoubleRow)
            nc.scalar.activation(s.PT[kp][:, :, :], pss[:, :, :], AF.Exp,
                                 scale=ISCALE)


def _denom(c, s, q):
    nc, sb, ps = c.nc, c.sb, c.ps
    s.rden = sb.tile([128, 1, CW], F32, tag="f32c", bufs=12, name=f"rden{q}")
    with nc.named_scope(f"denom{q}"):
        psd = ps.tile([128, 2, CW], F32, tag="mm", name=f"psd{q}")
        for j in range(NK // 2):
            nc.tensor.matmul(psd[:, 0, :], c.ones8[:], s.PT[j][:, :, :],
                             start=(j == 0), stop=(j == NK // 2 - 1),
                             perf_mode=PM.DoubleRow)
        nc.vector.reciprocal(s.rden[:, 0, :], psd[:, 0, :])


def _attnv(c, s, q, mids=None):
    nc, sb, ps = c.nc, c.sb, c.ps
    s.attnT = [sb.tile([128, 2, CW], FP8, tag="attc", bufs=12,
                       name=f"at{q}_{d}") for d in range(KD // 2)]
    with nc.named_scope(f"attnv{q}"):
        for dp in range(KD // 2):
            pa = ps.tile([128, 2, CW], F32, tag="mm", name=f"pa{q}_{dp}")
            for half in range(2):
                d = 2 * dp + half
                if mids and d in mids:
                    mids[d]()
                for j in range(NK // 2):
                    nc.tensor.matmul(pa[:, half, :],
                                     c.V[j][:, :, d * 128:(d + 1) * 128],
                                     s.PT[j][:, :, :],
                                     start=(j == 0), stop=(j == NK // 2 - 1),
                                     perf_mode=PM.DoubleRow)
            pall = pa[:, :, :]
            nc.vector.tensor_tensor(s.attnT[dp][:, :, :], pall,
                                    bc2(pall, s.rden[:, :, :]), op=OP.mult)


def _proj(c, s, q):
    nc, sb, ps = c.nc, c.sb, c.ps
    cs = slice(q * CW, (q + 1) * CW)
    xres = [sb.tile([128, 2, CW], F32, tag="f32p", bufs=11, name=f"xr{q}_{i}")
            for i in range(KD // 2)]
    for i in range(KD // 2):
        nc.sync.dma_start(xres[i][:], c.xres_d[i][:, :, cs])
    s.zp = [sb.tile([128, 2, CW], F32, tag="f32p", bufs=11, name=f"z{q}_{i}")
            for i in range(KD // 2)]
    s.z = [s.zp[e // 2][:, e % 2, :] for e in range(KD)]
    with nc.named_scope(f"proj{q}"):
        for ep in range(KD // 2):
            pp = ps.tile([128, 2, CW], F32, tag="mm", name=f"pp{q}_{ep}")
            for half in range(2):
                e = 2 * ep + half
                for d in range(KD // 2):
                    nc.tensor.matmul(pp[:, half, :],
                                     c.wp[d][:, :, e * 128:(e + 1) * 128],
                                     s.attnT[d][:, :, :],
                                     start=(d == 0), stop=(d == KD // 2 - 1),
                                     perf_mode=PM.DoubleRow)
            nc.vector.tensor_tensor(s.zp[ep][:, :, :], pp[:, :, :],
                                    xres[ep][:, :, :], op=OP.add)


def _ln_sums(c, s, q, which, z, zp):
    """sums: psum[:, 0, :] = mean(z), psum[:, 1, :] = mean(z^2), both
    partition-replicated after the ones-matmuls. Square runs paired on ACT,
    accumulations on Pool (z) / Pool (z^2)."""
    nc, sb, ps = c.nc, c.sb, c.ps
    with nc.named_scope(f"ln{which}s_{q}"):
        zacc = sb.tile([128, CW], F32, tag="acc2", bufs=4,
                       name=f"zacc{q}_{which}")
        sacc = sb.tile([128, CW], F32, tag="acc2", bufs=4,
                       name=f"sacc{q}_{which}")
        sq = [sb.tile([128, 2, CW], BF16, tag="sq", bufs=4,
                      name=f"sq{q}_{which}_{i}") for i in range(KD // 2)]
        sqv = [sq[e // 2][:, e % 2, :] for e in range(KD)]
        for i in range(KD // 2):
            nc.scalar.activation(sq[i][:, :, :], zp[i][:, :, :], AF.Square,
                                 scale=INV_SQD)
        for e in range(KD):
            if e == 1:
                nc.gpsimd.tensor_tensor(zacc[:], z[0], z[1], op=OP.add)
                nc.gpsimd.tensor_tensor(sacc[:], sqv[0], sqv[1], op=OP.add)
            elif e > 1:
                nc.gpsimd.tensor_tensor(zacc[:], zacc[:], z[e], op=OP.add)
                nc.gpsimd.tensor_tensor(sacc[:], sacc[:], sqv[e], op=OP.add)
        zb = sb.tile([128, CW], BF16, tag="zb", bufs=4,
                     name=f"zb{q}_{which}")
        sb2 = sb.tile([128, CW], BF16, tag="zb", bufs=4,
                      name=f"sb{q}_{which}")
        nc.vector.tensor_scalar(zb[:], zacc[:], INV_D, None, op0=OP.mult)
        nc.vector.tensor_copy(sb2[:], sacc[:])
    s.__setattr__(f"lnzb{which}", (zb, sb2))
    return None


def _ln_sums_mm(c, s, q, which):
    nc, ps = c.nc, c.ps
    zb, sb2 = getattr(s, f"lnzb{which}")
    pst = ps.tile([128, 2, CW], F32, tag="mm", name=f"pst{q}_{which}")
    with nc.named_scope(f"ln{which}m_{q}"):
        nc.tensor.matmul(pst[:, 0, :], c.ones[:], zb[:], start=True, stop=True)
        nc.tensor.matmul(pst[:, 1, :], c.ones[:], sb2[:],
                         start=True, stop=True)
    if which == 1:
        s.pst1 = pst
    else:
        s.pst2 = pst


def _ln_stats(c, s, q, which, pst):
    """From sums psum -> (Sv=mean*rstd, t0=rstd), both replicated."""
    nc, sb = c.nc, c.sb
    Sv = sb.tile([128, 1, CW], F32, tag="f32c", bufs=12, name=f"S{q}_{which}")
    t0 = sb.tile([128, 1, CW], F32, tag="f32c", bufs=12,
                 name=f"t0{q}_{which}")
    Svf, t0f = Sv[:, 0, :], t0[:, 0, :]
    with nc.named_scope(f"ln{which}st_{q}"):
        nc.scalar.activation(t0f, pst[:, 0, :], AF.Square)        # mu^2
        nc.vector.tensor_tensor(Svf, pst[:, 1, :], t0f,
                                op=OP.subtract)                   # var
        nc.scalar.activation(Svf, Svf, AF.Sqrt, bias=c.eps_t[:])
        nc.vector.reciprocal(t0f, Svf)                            # rstd
        nc.vector.tensor_tensor(Svf, pst[:, 0, :], t0f, op=OP.mult)
    return Sv, t0


def _ln1_sums(c, s, q):
    _ln_sums(c, s, q, 1, s.z, s.zp)


def _ln1_stats(c, s, q):
    _ln_sums_mm(c, s, q, 1)
    s.st1 = _ln_stats(c, s, q, 1, s.pst1)


def _ln1_apply(c, s, q):
    nc, sb = c.nc, c.sb
    Sv, t0 = s.st1
    s.hb = [sb.tile([128, 2, CW], FP8, tag="attc", bufs=12, name=f"hb{q}_{i}")
            for i in range(KD // 2)]
    with nc.named_scope(f"ln1a_{q}"):
        for i in range(KD // 2):
            zpi = s.zp[i][:, :, :]
            nc.vector.tensor_tensor(zpi, zpi, bc2(zpi, t0[:]), op=OP.mult)
            nc.vector.tensor_tensor(s.hb[i][:, :, :], zpi, bc2(zpi, Sv[:]),
                                    op=OP.subtract)
        # f32 residual carrier, x32-scaled (compensates W2's fp8 x32):
        # hres = 32*(z*rstd - mu*rstd) + 32*b2  (b2 arrives x32 from host)
        for i in range(KD // 2):
            zpi = s.zp[i][:, :, :]
            nc.gpsimd.tensor_tensor(zpi, zpi, bc2(zpi, Sv[:]),
                                    op=OP.subtract)
        for e in range(KD):
            nc.gpsimd.tensor_scalar(s.z[e], s.z[e], WSC,
                                    c.b2_t[:, e:e + 1],
                                    op0=OP.mult, op1=OP.add)
    s.hres = s.z


def _mlp1(c, s, q):
    nc, sb, ps = c.nc, c.sb, c.ps
    s.mlpb = [sb.tile([128, 2, CW], FP8, tag="mlpb", bufs=KH // 2,
                      name=f"mb{q}_{i}") for i in range(KH // 2)]
    with nc.named_scope(f"mlp1_{q}"):
        for h in range(KH):
            pm = ps.tile([128, 2, CW], F32, tag="mm", name=f"pm{q}_{h}")
            for k in range(KD // 2):
                nc.tensor.matmul(pm[:, 0, :],
                                 c.w1[k][:, :, h * 128:(h + 1) * 128],
                                 s.hb[k][:, :, :],
                                 start=(k == 0), stop=(k == KD // 2 - 1),
                                 perf_mode=PM.DoubleRow)
            nc.scalar.activation(s.mlpb[h // 2][:, h % 2, :], pm[:, 0, :],
                                 AF.Gelu, bias=c.b1_t[:, h:h + 1],
                                 scale=1.0 / WSC)


def _mlp2(c, s, q):
    nc, sb, ps = c.nc, c.sb, c.ps
    with nc.named_scope(f"mlp2_{q}"):
        for ep in range(KD // 2):
            p2 = ps.tile([128, 2, CW], F32, tag="mm", name=f"p2{q}_{ep}")
            for half in range(2):
                e = 2 * ep + half
                for k in range(KH // 2):
                    nc.tensor.matmul(p2[:, half, :],
                                     c.w2[k][:, :, e * 128:(e + 1) * 128],
                                     s.mlpb[k][:, :, :],
                                     start=(k == 0), stop=(k == KH // 2 - 1),
                                     perf_mode=PM.DoubleRow)
            nc.vector.tensor_tensor(s.zp[ep][:, :, :], p2[:, :, :],
                                    s.zp[ep][:, :, :], op=OP.add)
    s.z2 = s.hres


def _ln2_sums(c, s, q):
    _ln_sums(c, s, q, 2, s.z, s.zp)


def _ln2_stats(c, s, q):
    _ln_sums_mm(c, s, q, 2)
    s.st2 = _ln_stats(c, s, q, 2, s.pst2)


def _ln2_apply(c, s, q):
    nc = c.nc
    cs = slice(q * CW, (q + 1) * CW)
    Sv2, t02 = s.st2
    with nc.named_scope(f"ln2a_{q}"):
        for i in range(KD // 2):
            zpi = s.zp[i][:, :, :]
            eng = nc.vector if i == 0 else nc.gpsimd
            eng.tensor_tensor(zpi, zpi, bc2(zpi, t02[:]), op=OP.mult)
            eng.tensor_tensor(zpi, zpi, bc2(zpi, Sv2[:]), op=OP.subtract)
            nc.sync.dma_start(c.yt_d[i][:, :, cs], s.zp[i][:, :, :])


# ---------------- host side ----------------

def host_prep(inputs):
    """Returns per-core input maps (weights shared)."""
    x = np.asarray(inputs["x"], np.float32)
    B = x.shape[0]

    f8 = ml_dtypes.float8_e4m3

    def wtile8(w, kt, scale=1.0):  # fp8 DoubleRow pairs: [kt//2,128,2,out]
        wt = np.ascontiguousarray(np.asarray(w, np.float32).T * scale)
        t = wt.reshape(kt // 2, 2, 128, wt.shape[1]).transpose(0, 2, 1, 3)
        return np.ascontiguousarray(t).astype(f8)

    shared = {
        "wq": wtile8(inputs["Wq"], KD), "wk": wtile8(inputs["Wk"], KD),
        "wv": wtile8(inputs["Wv"], KD), "wp": wtile8(inputs["Wp"], KD),
        "w1": wtile8(inputs["W1"], KD, WSC),
        "w2": wtile8(inputs["W2"], KH, WSC),
        "bq": np.ascontiguousarray(
            np.asarray(inputs["bq"], np.float32).reshape(KD, 128).T),
        "bk": np.ascontiguousarray(
            np.asarray(inputs["bk"], np.float32).reshape(KD, 128).T),
        "b1": np.ascontiguousarray(
            np.asarray(inputs["b1"], np.float32).reshape(KH, 128).T),
        "b2": np.ascontiguousarray(
            (WSC * np.asarray(inputs["b2"], np.float32)).reshape(KD, 128).T),
    }
    bp = np.asarray(inputs["bp"], np.float32)
    wpbv = np.asarray(inputs["Wp"], np.float32) @ np.asarray(
        inputs["bv"], np.float32)
    per_core = []
    for b in range(B):
        xb_t = np.ascontiguousarray(x[b].T)          # [D, S]
        m = dict(shared)
        m["xt"] = np.ascontiguousarray(
            xb_t.reshape(KD // 2, 2, 128, S).transpose(0, 2, 1, 3)).astype(f8)
        xr = xb_t + (bp + wpbv)[:, None]
        m["xres"] = np.ascontiguousarray(
            xr.reshape(KD // 2, 2, 128, S).transpose(0, 2, 1, 3)
        ).astype(np.float32)
        per_core.append(m)
    return per_core


def assemble_output(results):
    """results: list of dicts with 'yt' [KD//2,128,2,S] -> [B,S,D] f32."""
    B = len(results)
    out = np.empty((B, S, D), np.float32)
    for b in range(B):
        yt = results[b]["yt"]                        # [KD//2, 128, 2, S]
        out[b] = yt.transpose(0, 2, 1, 3).reshape(D, S).T
    return out


# ------------------------------------------------------------------ kernel --
_CACHE = {}


def kernel(**inputs):
    install()  # birpatch
    from concourse.bass_utils import run_bass_kernel_spmd

    per_core = host_prep(inputs)
    n = len(per_core)
    key = "nc%d" % n
    if key not in _CACHE:
        _nc = bass.Bass("TRN2", target_bir_lowering=False, debug=False,
                        num_devices=n)
        build(_nc, reps=1)
        _CACHE[key] = _nc
    _nc = _CACHE[key]
    res = run_bass_kernel_spmd(_nc, per_core, list(range(n)), trace=False)
    return assemble_output(res.results)
